# revision 15
# baseline (speedup 1.0000x reference)
"""Trainium2 Bass kernel for nn_AttnFuser (fused MHA + FFN transformer block).

Sharding: 8 cores = 2 batches x 4 query-token slices of 512. Each core computes
the full block for its 512 query tokens; K/V projection over the full context
of its batch is replicated within each 4-core batch group (no collectives).

On-chip layout is feature-major ([feature, token]) for Q/K and the FFN, and
token-major for V. All large matmuls run in bf16 with fp32 PSUM accumulation.
Per-head QK layernorm stats are computed with block-ones PE matmuls and
broadcast back across partitions with indicator-matrix PE matmuls; RoPE's
half-rotation is a permutation-matrix PE matmul. The softmax denominator is
obtained for free by appending a ones-column to V (softmax rows sum to 1, so
the V bias is exactly additive after normalization).

SBUF is tight, so large tiles share pool tags in strict temporal chains
(e.g. the context tile's slot is later reused by the FFN hidden activations).
"""
import numpy as np
import ml_dtypes

BF16 = ml_dtypes.bfloat16

D, T, M, H, DH, DFF = 1024, 512, 2048, 16, 64, 4096
NCH = D // 128      # 8 feature chunks
TTK = M // 512      # 4 context token tiles
MC = M // 128       # 16 context chunks
B, N = 2, 2048      # full problem dims

_BUILT = {}


def _patch_tile_drain():
    """This walrus build rejects >1 sem wait on an InstDrain (TPB_CTRL
    setupSyncWait). Split the TileContext tail-drain waits onto nop insts."""
    import concourse.tile as tile_mod
    from concourse import mybir
    from concourse.vector_clock import ScopedClock
    if getattr(tile_mod.TileContext, "_drain_patched", False):
        return

    def _drain_and_barrier(self, tick_clock, wait_clock):
        nc = self.nc
        drain_inst = nc.sync.drain()
        wait_clock.add_sem_waits(
            drain_inst.ins, ScopedClock({None: tick_clock.global_clock}))
        si = drain_inst.ins.sync_info
        waits = list(si.on_wait or []) if si else []
        if len(waits) > 1:
            drain_inst.ins.sync_info = mybir.SyncInfo(
                on_wait=waits[:1], on_update=list(si.on_update or []))
            for w in waits[1:]:
                nop = nc.sync.nop(nofuse=True, hint="split_drain_wait")
                nop.ins.sync_info = mybir.SyncInfo(on_wait=[w], on_update=[])
        nc.all_engine_barrier()
        popped = nc._tile_sem_poison_stack.pop()
        assert popped is self._sem_poison
        nc.clear_and_free_semaphores(list(self.sems.allocated().values()))
        nc.all_engine_barrier()

    tile_mod.TileContext._drain_and_barrier = _drain_and_barrier
    tile_mod.TileContext._drain_patched = True


def _split_sync_waits(nc, max_waits=1):
    """This walrus build rejects instructions carrying more than ~1 sem wait
    (setupSyncWait: 'Too many sync wait commands'). Hoist extra waits onto
    same-engine NOPs inserted immediately before the instruction — the engine
    executes them in order, so all waits are still satisfied before the op."""
    from concourse import mybir
    n = 0
    for f in nc.m.functions:
        for bb in f.blocks:
            insts = bb.instructions
            new = []
            for inst in insts:
                si = getattr(inst, "sync_info", None)
                waits = list(si.on_wait) if si and si.on_wait else []
                if len(waits) > max_waits:
                    for w in waits[max_waits:]:
                        nop = mybir.InstNoOp(
                            name=f"wsplit_{n}",
                            sync_info=mybir.SyncInfo(on_wait=[w], on_update=[]),
                            bass_nofuse=True,
                            engine=inst.engine,
                        )
                        nc.register_instruction(nop)
                        n += 1
                        new.append(nop)
                    inst.sync_info = mybir.SyncInfo(
                        on_wait=waits[:max_waits],
                        on_update=list(si.on_update or []))
                new.append(inst)
            insts[:] = new
    return n


def input_specs(with_tq, with_tk):
    """(name, shape, np_dtype) for every DRAM input."""
    sp = [
        ("xq", (D, T), BF16), ("xqf", (D, T), np.float32),
        ("xc", (D, M), BF16),
        ("wq", (D, D), BF16), ("wk", (D, D), BF16), ("wv", (D, D), BF16),
        ("w1", (D, DFF), BF16), ("w2", (DFF, D), BF16),
        ("bq", (D,), np.float32), ("bk", (D,), np.float32),
        ("bv", (D,), np.float32), ("b1", (DFF,), np.float32),
        ("b2", (D,), np.float32),
        ("fng", (D,), np.float32), ("fnb", (D,), np.float32),
        ("cq2", (128, T), BF16), ("sq2", (128, T), BF16), ("csq2", (128, T), BF16),
        ("ck2", (128, M), BF16), ("sk2", (128, M), BF16), ("csk2", (128, M), BF16),
        ("bo16", (128, 8 * 16), BF16),
        ("indall", (16, D), BF16),
        ("perm", (128, 128), BF16),
        ("id64", (64, 64), BF16),
        ("onesb", (1, 128), np.float32),     # fp32 ones row (recip broadcast lhsT)
        ("ones128f", (128, 1), np.float32),  # value 1/1024 (FFN stats lhsT)
        ("onesr", (1, 128), BF16),           # bf16 ones row (FFN bc lhsT)
    ]
    if with_tq:
        sp.append(("tq", (128, T), BF16))
    if with_tk:
        sp.append(("tk", (128, M), BF16))
    return sp


def build(with_tq=False, with_tk=False):
    import concourse.bass as bass
    import concourse.mybir as mybir
    import concourse.tile as tile
    from contextlib import ExitStack

    _patch_tile_drain()
    AF = mybir.ActivationFunctionType
    OP = mybir.AluOpType
    bf = mybir.dt.bfloat16
    f32 = mybir.dt.float32

    nc = bass.Bass()
    dram = {}
    for name, shape, npdt in input_specs(with_tq, with_tk):
        dt = bf if npdt is BF16 else f32
        dram[name] = nc.declare_dram_parameter(name, list(shape), dt, isOutput=False)
    out_d = nc.declare_dram_parameter("out", [D, T], f32, isOutput=True)

    with tile.TileContext(nc) as tc, ExitStack() as ctx:
        const = ctx.enter_context(tc.tile_pool(name="const", bufs=1))
        u = ctx.enter_context(tc.tile_pool(name="u", bufs=1))
        wt = ctx.enter_context(tc.tile_pool(name="wt", bufs=3))
        pmm = ctx.enter_context(tc.tile_pool(name="pmm", bufs=4, space="PSUM"))
        pstat = ctx.enter_context(tc.tile_pool(name="pstat", bufs=2, space="PSUM"))
        pot = ctx.enter_context(tc.tile_pool(name="pot", bufs=2, space="PSUM"))

        def load(pool, name, rearr=None, tag=None, **kw):
            src = dram[name][:]
            if rearr is not None:
                src = src.rearrange(rearr, **kw)
            t = pool.tile(list(src.shape), src.dtype, tag=tag or name, name=name)
            nc.sync.dma_start(out=t[:], in_=src)
            return t

        # small constants
        bo16 = load(const, "bo16")
        indall = load(const, "indall")
        perm = load(const, "perm")
        id64 = load(const, "id64")
        onesb = load(const, "onesb")
        ones128f = load(const, "ones128f")
        onesr = load(const, "onesr")
        bq_sb = load(const, "bq", "(c p) -> p c", p=128)
        bk_sb = load(const, "bk", "(c p) -> p c", p=128)
        bv_sb = load(const, "bv", "(c p) -> p c", p=128)
        b2_sb = load(const, "b2", "(c p) -> p c", p=128)
        b1_sb = load(const, "b1", "(c p) -> p c", p=128)
        fng_sb = load(const, "fng", "(c p) -> p c", p=128)
        fnb_sb = load(const, "fnb", "(c p) -> p c", p=128)
        eps = const.tile([128, 1], f32, tag="eps", name="eps")
        nc.vector.memset(eps[:], 1e-5)

        # rope tables — their tag slots are later reused by attention/FFN tiles
        cq2 = load(u, "cq2"); sq2 = load(u, "sq2"); csq2 = load(u, "csq2")
        ck2 = load(u, "ck2"); sk2 = load(u, "sk2"); csk2 = load(u, "csk2")
        tqt = load(const, "tq") if with_tq else None
        tkt = load(const, "tk") if with_tk else None

        # big activations (tags are temporal chains — comments show the chain)
        xc_sb = u.tile([128, NCH, M], bf, tag="cA", name="xc")       # cA: xc->xqf->h1
        nc.sync.dma_start(out=xc_sb[:], in_=dram["xc"][:].rearrange("(c p) m -> p c m", p=128))
        xq_sb = u.tile([128, NCH, T], bf, tag="cE", name="xqbf")     # cE: xq->h
        nc.sync.dma_start(out=xq_sb[:], in_=dram["xq"][:].rearrange("(c p) n -> p c n", p=128))
        wv_sb = u.tile([128, NCH, D], bf, tag="cD", name="wvsb")     # cD: wv->OT
        nc.sync.dma_start(out=wv_sb[:], in_=dram["wv"][:].rearrange("(k p) m -> p k m", p=128))

        KT = u.tile([128, NCH, M], bf, tag="cB", name="KT")          # cB: KT->o
        QT = u.tile([128, NCH, T], bf, tag="cQT", name="QT")
        V = u.tile([128, MC, H, DH + 1], bf, tag="cC", name="V")     # cC: V->x2f
        aK = u.tile([16, M], bf, tag="caK", name="aK")               # caK: aK->dsb
        bK = u.tile([16, M], bf, tag="cbK", name="bK")               # cbK: bK->rsb
        aQ = u.tile([16, T], bf, tag="caQ", name="aQ")               # caQ: aQ->af
        bQ = u.tile([16, T], bf, tag="cbQ", name="bQ")               # cbQ: bQ->bff

        # V ones-columns (col DH for every head)
        nc.vector.memset(V[:, :, :, DH:DH + 1], 1.0)

        # ---------------- projections ----------------
        def proj_featmajor(wname, x_sb, ntt, out_t, bias):
            wr = dram[wname][:].rearrange("(k p) m -> p k m", p=128)
            for m in range(NCH):
                wtile = wt.tile([128, NCH, 128], bf, tag="wqk", name=f"w_{wname}_{m}")
                nc.sync.dma_start(out=wtile[:], in_=wr[:, :, m * 128:(m + 1) * 128])
                for n in range(ntt):
                    ps = pmm.tile([128, 512], f32, tag="pmm", name=f"ps_{wname}_{m}_{n}")
                    for kc in range(NCH):
                        nc.tensor.matmul(ps[:], lhsT=wtile[:, kc, :],
                                         rhs=x_sb[:, kc, n * 512:(n + 1) * 512],
                                         start=(kc == 0), stop=(kc == NCH - 1))
                    nc.scalar.activation(out=out_t[:, m, n * 512:(n + 1) * 512],
                                         in_=ps[:], func=AF.Identity,
                                         bias=bias[:, m:m + 1], scale=1.0)

        proj_featmajor("wk", xc_sb, TTK, KT, bk_sb)
        proj_featmajor("wq", xq_sb, 1, QT, bq_sb)

        # V token-major (no bias: the V bias is exactly additive after softmax)
        for mc in range(MC):
            for n in range(2):
                ps = pmm.tile([128, 512], f32, tag="pmm", name=f"ps_v_{mc}_{n}")
                for kc in range(NCH):
                    nc.tensor.matmul(ps[:], lhsT=xc_sb[:, kc, mc * 128:(mc + 1) * 128],
                                     rhs=wv_sb[:, kc, n * 512:(n + 1) * 512],
                                     start=(kc == 0), stop=(kc == NCH - 1))
                pv = ps[:].rearrange("p (hh d) -> p hh d", d=DH)
                nc.scalar.activation(out=V[:, mc, 8 * n:8 * n + 8, 0:DH],
                                     in_=pv[:], func=AF.Copy)

        # xqf (residual) loads into the xc slot once projections are done;
        # OT takes over wv's slot
        xqf_sb = u.tile([128, NCH, T], f32, tag="cA", name="xqf")
        nc.sync.dma_start(out=xqf_sb[:], in_=dram["xqf"][:].rearrange("(c p) n -> p c n", p=128))
        OT = u.tile([128, NCH, T], bf, tag="cD", name="OT")

        # ---------------- per-head QK layernorm + rope ----------------
        def ln_stats(X, ntt, a_sb, b_sb, pref):
            for tt in range(ntt):
                ts_ = slice(tt * 512, (tt + 1) * 512)
                sx = pstat.tile([16, 512], f32, tag="pstat", name=f"sx_{pref}{tt}")
                sq = pstat.tile([16, 512], f32, tag="pstat", name=f"sq_{pref}{tt}")
                for c in range(NCH):
                    xs = X[:, c, ts_]
                    x2 = u.tile([128, 512], bf, tag="x2", bufs=2, name=f"x2_{pref}{tt}_{c}")
                    nc.vector.tensor_mul(out=x2[:], in0=xs, in1=xs)
                    # one-hot block lhsT accumulates chunk c's two head rows;
                    # the other 14 rows accumulate zeros
                    nc.tensor.matmul(sx[:], lhsT=bo16[:, c * 16:(c + 1) * 16], rhs=xs,
                                     start=(c == 0), stop=(c == NCH - 1),
                                     skip_group_check=True)
                    nc.tensor.matmul(sq[:], lhsT=bo16[:, c * 16:(c + 1) * 16], rhs=x2[:],
                                     start=(c == 0), stop=(c == NCH - 1),
                                     skip_group_check=True)
                mu = u.tile([16, 512], f32, tag="cmu", name=f"mu_{pref}{tt}")
                nc.scalar.activation(out=mu[:], in_=sx[:], func=AF.Copy)
                t1 = u.tile([16, 512], f32, tag="ct1", name=f"t1_{pref}{tt}")
                nc.vector.tensor_mul(out=t1[:], in0=mu[:], in1=mu[:])
                var = u.tile([16, 512], f32, tag="cvar", name=f"var_{pref}{tt}")
                nc.vector.tensor_tensor(out=var[:], in0=sq[:], in1=t1[:], op=OP.subtract)
                sd = u.tile([16, 512], f32, tag="csd", name=f"sd_{pref}{tt}")
                nc.scalar.activation(out=sd[:], in_=var[:], func=AF.Sqrt,
                                     bias=eps[0:16, :], scale=1.0)
                with nc.allow_low_precision("bf16 rstd for broadcast matmul"):
                    nc.vector.reciprocal(out=a_sb[:, ts_], in_=sd[:])
                nc.vector.tensor_mul(out=b_sb[:, ts_], in0=mu[:], in1=a_sb[:, ts_])

        def ln_rope(X, ntt, a_sb, b_sb, ctab, stab, cstab, ttab, pref):
            Nw = ntt * 512
            for c in range(NCH):
                zz1 = u.tile([128, M], bf, tag="czz1", name=f"zz1_{pref}{c}")
                zz2 = u.tile([128, M], bf, tag="czz2", name=f"zz2_{pref}{c}")
                A = u.tile([128, M], bf, tag="cAbc", name=f"A_{pref}{c}")
                Bt = u.tile([128, M], bf, tag="cBbc", name=f"B_{pref}{c}")
                for tt in range(ntt):
                    ts_ = slice(tt * 512, (tt + 1) * 512)
                    aps = pmm.tile([128, 512], f32, tag="pmm", name=f"aps_{pref}{c}_{tt}")
                    nc.tensor.matmul(aps[:], lhsT=indall[:, c * 128:(c + 1) * 128],
                                     rhs=a_sb[:, ts_], start=True, stop=True)
                    nc.scalar.activation(out=A[:, ts_], in_=aps[:], func=AF.Copy)
                    bps = pmm.tile([128, 512], f32, tag="pmm", name=f"bps_{pref}{c}_{tt}")
                    nc.tensor.matmul(bps[:], lhsT=indall[:, c * 128:(c + 1) * 128],
                                     rhs=b_sb[:, ts_], start=True, stop=True)
                    nc.scalar.activation(out=Bt[:, ts_], in_=bps[:], func=AF.Copy)
                    rot = pmm.tile([128, 512], f32, tag="pmm", name=f"rot_{pref}{c}_{tt}")
                    nc.tensor.matmul(rot[:], lhsT=perm[:], rhs=X[:, c, ts_],
                                     start=True, stop=True)
                    nc.vector.tensor_mul(out=zz2[:, ts_], in0=rot[:], in1=stab[:, ts_])
                # out = A*(C2*x + S2*rot) - B*(C2+S2) [+ Tadd]
                nc.vector.tensor_mul(out=zz1[:, :Nw], in0=X[:, c, :Nw], in1=ctab[:, :Nw])
                nc.vector.tensor_add(out=zz1[:, :Nw], in0=zz1[:, :Nw], in1=zz2[:, :Nw])
                nc.vector.tensor_mul(out=zz1[:, :Nw], in0=zz1[:, :Nw], in1=A[:, :Nw])
                nc.vector.tensor_mul(out=zz2[:, :Nw], in0=Bt[:, :Nw], in1=cstab[:, :Nw])
                nc.vector.tensor_tensor(out=X[:, c, :Nw], in0=zz1[:, :Nw],
                                        in1=zz2[:, :Nw], op=OP.subtract)
                if ttab is not None:
                    nc.vector.tensor_add(out=X[:, c, :Nw], in0=X[:, c, :Nw],
                                         in1=ttab[:, :Nw])

        ln_stats(KT, TTK, aK, bK, "k")
        ln_rope(KT, TTK, aK, bK, ck2, sk2, csk2, tkt, "k")
        ln_stats(QT, 1, aQ, bQ, "q")
        ln_rope(QT, 1, aQ, bQ, cq2, sq2, csq2, tqt, "q")

        # ---------------- attention (per head, ctx in waves of 4 chunks) ----
        WV = 4                      # ctx chunks per wave
        for h in range(H):
            c, p0 = h // 2, 64 * (h % 2)
            ot = pot.tile([128, 512], f32, tag="pot", name=f"ot_{h}")
            for w in range(MC // WV):
                att = u.tile([128, WV, 512], bf,
                             tag=("ck2" if w % 2 == 0 else "sk2"), name=f"att_{h}_{w}")
                for i in range(WV):
                    mc = w * WV + i
                    sps = pmm.tile([128, 512], f32, tag="pmm", name=f"sps_{h}_{mc}")
                    nc.tensor.matmul(sps[:], lhsT=KT[p0:p0 + 64, c, mc * 128:(mc + 1) * 128],
                                     rhs=QT[p0:p0 + 64, c, :], start=True, stop=True)
                    nc.scalar.activation(out=att[:, i, :], in_=sps[:], func=AF.Exp,
                                         scale=0.125)
                    nc.tensor.matmul(ot[0:DH + 1, :], lhsT=V[:, mc, h, :],
                                     rhs=att[:, i, :], start=(mc == 0),
                                     stop=(mc == MC - 1), skip_group_check=True)
            # denominator (row 64) -> reciprocal at partition 0 -> broadcast
            dsb = u.tile([128, 512], f32, tag="caK", name=f"dsb_{h}")
            nc.scalar.activation(out=dsb[64:65, :], in_=ot[64:65, :], func=AF.Copy)
            d0 = u.tile([1, 512], f32, tag="cd0", bufs=2, name=f"d0_{h}")
            nc.sync.dma_start(out=d0[:], in_=dsb[64:65, :])
            r0 = u.tile([1, 512], f32, tag="cr0", bufs=2, name=f"r0_{h}")
            nc.vector.reciprocal(out=r0[:], in_=d0[:])
            rps = pmm.tile([128, 512], f32, tag="pmm", name=f"rps_{h}")
            nc.tensor.matmul(rps[:], lhsT=onesb[:], rhs=r0[:], start=True, stop=True)
            rsb = u.tile([128, 512], f32, tag="cbK", name=f"rsb_{h}")
            nc.scalar.activation(out=rsb[p0:p0 + 64, :], in_=rps[p0:p0 + 64, :],
                                 func=AF.Copy)
            if p0 == 0:
                nc.vector.tensor_mul(out=OT[0:64, c, :], in0=ot[0:64, :],
                                     in1=rsb[0:64, :])
            else:
                # odd head: O sits at PSUM rows 0..63 but belongs at partitions
                # 64..127 of OT; shift with an identity matmul (PE can cross
                # partitions, DVE/ACT cannot)
                tmp = u.tile([128, 512], bf, tag="cotmp", bufs=2, name=f"otmp_{h}")
                nc.scalar.activation(out=tmp[0:64, :], in_=ot[0:64, :], func=AF.Copy)
                ps2 = pmm.tile([128, 512], f32, tag="pmm", name=f"ps2_{h}")
                nc.tensor.matmul(ps2[64:128, :], lhsT=id64[:], rhs=tmp[0:64, :],
                                 start=True, stop=True)
                nc.vector.tensor_mul(out=OT[64:128, c, :], in0=ps2[64:128, :],
                                     in1=rsb[64:128, :])

        # ---------------- residual + FFN ----------------
        o_sb = u.tile([128, NCH, T], f32, tag="cB", name="o")
        for c in range(NCH):
            nc.vector.tensor_add(out=o_sb[:, c, :], in0=xqf_sb[:, c, :], in1=OT[:, c, :])
            nc.vector.tensor_scalar_add(out=o_sb[:, c, :], in0=o_sb[:, c, :],
                                        scalar1=bv_sb[:, c:c + 1])

        # FFN layernorm over all 1024 features: fp32 ones-matmul stats
        x2f = u.tile([128, NCH, T], f32, tag="cC", name="x2f")
        for c in range(NCH):
            nc.vector.tensor_mul(out=x2f[:, c, :], in0=o_sb[:, c, :], in1=o_sb[:, c, :])
        smean = pstat.tile([16, 512], f32, tag="pstat", name="smean")
        for c in range(NCH):
            nc.tensor.matmul(smean[0:1, :], lhsT=ones128f[:], rhs=o_sb[:, c, :],
                             start=(c == 0), stop=(c == NCH - 1))
        smsq = pstat.tile([16, 512], f32, tag="pstat", name="smsq")
        for c in range(NCH):
            nc.tensor.matmul(smsq[0:1, :], lhsT=ones128f[:], rhs=x2f[:, c, :],
                             start=(c == 0), stop=(c == NCH - 1))
        muf = u.tile([1, 512], f32, tag="cmu", name="muf")
        nc.scalar.activation(out=muf[:], in_=smean[0:1, :], func=AF.Copy)
        t1f = u.tile([1, 512], f32, tag="ct1", name="t1f")
        nc.vector.tensor_mul(out=t1f[:], in0=muf[:], in1=muf[:])
        varf = u.tile([1, 512], f32, tag="cvar", name="varf")
        nc.vector.tensor_tensor(out=varf[:], in0=smsq[0:1, :], in1=t1f[:], op=OP.subtract)
        sdf = u.tile([1, 512], f32, tag="csd", name="sdf")
        nc.scalar.activation(out=sdf[:], in_=varf[:], func=AF.Sqrt, bias=eps[0:1, :],
                             scale=1.0)
        af = u.tile([1, 512], bf, tag="caQ", name="af")
        with nc.allow_low_precision("bf16 rstd for broadcast matmul"):
            nc.vector.reciprocal(out=af[:], in_=sdf[:])
        bff = u.tile([1, 512], bf, tag="cbQ", name="bff")
        nc.vector.tensor_mul(out=bff[:], in0=muf[:], in1=af[:])
        a2ps = pmm.tile([128, 512], f32, tag="pmm", name="a2ps")
        nc.tensor.matmul(a2ps[:], lhsT=onesr[:], rhs=af[:], start=True, stop=True)
        A2 = u.tile([128, 512], bf, tag="cq2", name="A2")
        nc.scalar.activation(out=A2[:], in_=a2ps[:], func=AF.Copy)
        b2ps = pmm.tile([128, 512], f32, tag="pmm", name="b2ps")
        nc.tensor.matmul(b2ps[:], lhsT=onesr[:], rhs=bff[:], start=True, stop=True)
        B2 = u.tile([128, 512], bf, tag="sq2", name="B2")
        nc.scalar.activation(out=B2[:], in_=b2ps[:], func=AF.Copy)

        h_sb = u.tile([128, NCH, T], bf, tag="cE", name="hsb")
        for c in range(NCH):
            tn = u.tile([128, 512], f32, tag="csk2", name=f"tn_{c}")
            nc.vector.tensor_mul(out=tn[:], in0=o_sb[:, c, :], in1=A2[:])
            nc.vector.tensor_tensor(out=tn[:], in0=tn[:], in1=B2[:], op=OP.subtract)
            nc.vector.tensor_scalar(out=h_sb[:, c, :], in0=tn[:],
                                    scalar1=fng_sb[:, c:c + 1],
                                    scalar2=fnb_sb[:, c:c + 1],
                                    op0=OP.mult, op1=OP.add)

        # FFN matmul 1 + exact GELU
        h1_sb = u.tile([128, DFF // 128, T], bf, tag="cA", name="h1")
        w1r = dram["w1"][:].rearrange("(k p) m -> p k m", p=128)
        for m in range(DFF // 128):
            w1t = wt.tile([128, NCH, 128], bf, tag="wqk", name=f"w1t_{m}")
            nc.sync.dma_start(out=w1t[:], in_=w1r[:, :, m * 128:(m + 1) * 128])
            ps = pmm.tile([128, 512], f32, tag="pmm", name=f"ps_h1_{m}")
            for kc in range(NCH):
                nc.tensor.matmul(ps[:], lhsT=w1t[:, kc, :], rhs=h_sb[:, kc, :],
                                 start=(kc == 0), stop=(kc == NCH - 1))
            nc.scalar.activation(out=h1_sb[:, m, :], in_=ps[:], func=AF.Gelu,
                                 bias=b1_sb[:, m:m + 1], scale=1.0)

        # FFN matmul 2 + bias + residual (w2 streamed as two half-K tiles that
        # reuse the ck2/csk2 table slots)
        w2r = dram["w2"][:].rearrange("(k p) m -> p k m", p=128)
        KH = DFF // 128 // 2        # 16 k-chunks per half
        for m in range(NCH):
            w2a = u.tile([128, KH, 128], bf, tag="ck2", name=f"w2a_{m}")
            nc.sync.dma_start(out=w2a[:], in_=w2r[:, 0:KH, m * 128:(m + 1) * 128])
            w2b = u.tile([128, KH, 128], bf, tag="csk2", name=f"w2b_{m}")
            nc.sync.dma_start(out=w2b[:], in_=w2r[:, KH:2 * KH, m * 128:(m + 1) * 128])
            ps = pmm.tile([128, 512], f32, tag="pmm", name=f"ps_h2_{m}")
            for kc in range(2 * KH):
                wsl = w2a[:, kc, :] if kc < KH else w2b[:, kc - KH, :]
                nc.tensor.matmul(ps[:], lhsT=wsl, rhs=h1_sb[:, kc, :],
                                 start=(kc == 0), stop=(kc == 2 * KH - 1))
            nc.vector.tensor_add(out=o_sb[:, m, :], in0=ps[:], in1=o_sb[:, m, :])
            nc.vector.tensor_scalar_add(out=o_sb[:, m, :], in0=o_sb[:, m, :],
                                        scalar1=b2_sb[:, m:m + 1])
            nc.sync.dma_start(
                out=out_d[:].rearrange("(c p) n -> p c n", p=128)[:, m, :],
                in_=o_sb[:, m, :])

    _split_sync_waits(nc)
    return nc


# ---------------------------------------------------------------- host side

def _rope_tables(pos, g, b_ln):
    """Feature-major rope coefficient tiles [128, N] (pattern repeats per 64).

    out = C2*z + S2*rot(z) + Tadd with z the per-head layernormed vector,
    C2 = C*G[p], S2 = S*G[rp], Tadd = C*B[p] + S*B[rp].
    """
    half = DH // 2
    inv = (1.0 / (10000.0 ** (np.arange(half, dtype=np.float32) / half))).astype(np.float32)
    ang = pos.astype(np.float32)[None, :] * inv[:, None]          # [32, N]
    c = np.cos(ang).astype(np.float32)
    s = np.sin(ang).astype(np.float32)
    C64 = np.concatenate([c, c], axis=0)                          # [64, N]
    S64 = np.concatenate([-s, s], axis=0)
    G = np.ones(DH, np.float32) if g is None else np.asarray(g, np.float32)
    Bv = np.zeros(DH, np.float32) if b_ln is None else np.asarray(b_ln, np.float32)
    rp = np.concatenate([np.arange(32, 64), np.arange(0, 32)])
    C2 = C64 * G[:, None]
    S2 = S64 * G[rp][:, None]
    CS2 = C2 + S2
    Tadd = C64 * Bv[:, None] + S64 * Bv[rp][:, None]
    tile = lambda X: np.concatenate([X, X], axis=0)               # [128, N]
    has_t = bool(np.abs(Bv).max() > 0)
    return (tile(C2).astype(BF16), tile(S2).astype(BF16), tile(CS2).astype(BF16),
            tile(Tadd).astype(BF16) if has_t else None)


def _consts():
    bo16 = np.zeros((128, 8, 16), np.float32)
    for c in range(NCH):
        for pp in range(128):
            bo16[pp, c, 2 * c + (pp >= 64)] = 1.0 / DH
    bo16 = bo16.reshape(128, 8 * 16)
    indall = np.zeros((16, D), np.float32)
    for c in range(NCH):
        for pp in range(128):
            indall[2 * c + (pp >= 64), c * 128 + pp] = 1.0
    perm = np.zeros((128, 128), np.float32)
    for mm in range(128):
        k = (mm // 64) * 64 + ((mm % 64) + 32) % 64
        perm[k, mm] = 1.0
    return {
        "bo16": bo16.astype(BF16),
        "indall": indall.astype(BF16),
        "perm": perm.astype(BF16),
        "id64": np.eye(64, dtype=np.float32).astype(BF16),
        "onesb": np.ones((1, 128), np.float32),
        "ones128f": np.full((128, 1), 1.0 / D, np.float32),
        "onesr": np.ones((1, 128), BF16),
    }


def make_in_maps(inputs):
    """Full inputs -> (per-core input dicts, build flags)."""
    inputs = {k: np.asarray(v) for k, v in inputs.items()}
    consts = _consts()
    shared = {
        "wq": inputs["Wq"].astype(BF16), "wk": inputs["Wk"].astype(BF16),
        "wv": inputs["Wv"].astype(BF16), "w1": inputs["W1"].astype(BF16),
        "w2": inputs["W2"].astype(BF16),
        "bq": inputs["bq"].astype(np.float32), "bk": inputs["bk"].astype(np.float32),
        "bv": inputs["bv"].astype(np.float32), "b1": inputs["b1"].astype(np.float32),
        "b2": inputs["b2"].astype(np.float32),
        "fng": inputs["fn_g"].astype(np.float32), "fnb": inputs["fn_b"].astype(np.float32),
        **consts,
    }
    in_maps = []
    with_tq = with_tk = False
    for core in range(8):
        b, t0 = core // 4, (core % 4) * T
        xqf = np.ascontiguousarray(inputs["query"][b, t0:t0 + T].T).astype(np.float32)
        cq, sq, csq, tq = _rope_tables(inputs["qpos"][b, t0:t0 + T],
                                       inputs["qn_g"], inputs["qn_b"])
        ck, sk, csk, tk = _rope_tables(inputs["cpos"][b],
                                       inputs["kn_g"], inputs["kn_b"])
        m = dict(shared)
        m.update({
            "xqf": xqf, "xq": xqf.astype(BF16),
            "xc": np.ascontiguousarray(inputs["context"][b].T).astype(BF16),
            "cq2": cq, "sq2": sq, "csq2": csq,
            "ck2": ck, "sk2": sk, "csk2": csk,
        })
        if tq is not None:
            m["tq"] = tq
            with_tq = True
        if tk is not None:
            m["tk"] = tk
            with_tk = True
        in_maps.append(m)
    return in_maps, with_tq, with_tk


def kernel(**inputs):
    from concourse.bass_utils import run_bass_kernel_spmd
    in_maps, with_tq, with_tk = make_in_maps(inputs)
    key = (with_tq, with_tk)
    if key not in _BUILT:
        _BUILT[key] = build(*key)
    nc = _BUILT[key]
    res = run_bass_kernel_spmd(nc, in_maps, core_ids=list(range(8)))
    out = np.zeros((B, N, D), np.float32)
    for core in range(8):
        b, t0 = core // 4, (core % 4) * T
        out[b, t0:t0 + T] = res.results[core]["out"].T
    return out


# revision 17
# speedup vs baseline: 1.1673x; 1.1673x over previous
"""Trainium2 Bass kernel for nn_AttnFuser (fused MHA + FFN transformer block).

Sharding: 8 cores = 2 batches x 4 query-token slices of 512. Each core computes
the full block for its 512 query tokens; K/V projection over the full context
of its batch is replicated within each 4-core batch group (no collectives).

On-chip layout is feature-major ([feature, token]) for Q/K and the FFN, and
token-major for V. All large matmuls run in bf16 with fp32 PSUM accumulation.
Per-head QK layernorm stats are computed with block-ones PE matmuls and
broadcast back across partitions with indicator-matrix PE matmuls; RoPE's
half-rotation is a permutation-matrix PE matmul. The softmax denominator is
obtained for free by appending a ones-column to V (softmax rows sum to 1, so
the V bias is exactly additive after normalization).

SBUF is tight, so large tiles share pool tags in strict temporal chains
(e.g. the context tile's slot is later reused by the FFN hidden activations).
"""
import numpy as np
import ml_dtypes

BF16 = ml_dtypes.bfloat16

D, T, M, H, DH, DFF = 1024, 512, 2048, 16, 64, 4096
NCH = D // 128      # 8 feature chunks
TTK = M // 512      # 4 context token tiles
MC = M // 128       # 16 context chunks
B, N = 2, 2048      # full problem dims

_BUILT = {}


def _patch_tile_drain():
    """This walrus build rejects >1 sem wait on an InstDrain (TPB_CTRL
    setupSyncWait). Split the TileContext tail-drain waits onto nop insts."""
    import concourse.tile as tile_mod
    from concourse import mybir
    from concourse.vector_clock import ScopedClock
    if getattr(tile_mod.TileContext, "_drain_patched", False):
        return

    def _drain_and_barrier(self, tick_clock, wait_clock):
        nc = self.nc
        drain_inst = nc.sync.drain()
        wait_clock.add_sem_waits(
            drain_inst.ins, ScopedClock({None: tick_clock.global_clock}))
        si = drain_inst.ins.sync_info
        waits = list(si.on_wait or []) if si else []
        if len(waits) > 1:
            drain_inst.ins.sync_info = mybir.SyncInfo(
                on_wait=waits[:1], on_update=list(si.on_update or []))
            for w in waits[1:]:
                nop = nc.sync.nop(nofuse=True, hint="split_drain_wait")
                nop.ins.sync_info = mybir.SyncInfo(on_wait=[w], on_update=[])
        nc.all_engine_barrier()
        popped = nc._tile_sem_poison_stack.pop()
        assert popped is self._sem_poison
        nc.clear_and_free_semaphores(list(self.sems.allocated().values()))
        nc.all_engine_barrier()

    tile_mod.TileContext._drain_and_barrier = _drain_and_barrier
    tile_mod.TileContext._drain_patched = True


def _split_sync_waits(nc, max_waits=1):
    """This walrus build rejects instructions carrying more than ~1 sem wait
    (setupSyncWait: 'Too many sync wait commands'). Hoist extra waits onto
    same-engine NOPs inserted immediately before the instruction — the engine
    executes them in order, so all waits are still satisfied before the op."""
    from concourse import mybir
    n = 0
    for f in nc.m.functions:
        for bb in f.blocks:
            insts = bb.instructions
            new = []
            for inst in insts:
                si = getattr(inst, "sync_info", None)
                waits = list(si.on_wait) if si and si.on_wait else []
                if len(waits) > max_waits:
                    for w in waits[max_waits:]:
                        nop = mybir.InstNoOp(
                            name=f"wsplit_{n}",
                            sync_info=mybir.SyncInfo(on_wait=[w], on_update=[]),
                            bass_nofuse=True,
                            engine=inst.engine,
                        )
                        nc.register_instruction(nop)
                        n += 1
                        new.append(nop)
                    inst.sync_info = mybir.SyncInfo(
                        on_wait=waits[:max_waits],
                        on_update=list(si.on_update or []))
                new.append(inst)
            insts[:] = new
    return n


def input_specs(with_tq, with_tk):
    """(name, shape, np_dtype) for every DRAM input."""
    sp = [
        ("xq", (D, T), BF16), ("xqf", (D, T), np.float32),
        ("xc", (D, M), BF16),
        ("wq", (D, D), BF16), ("wk", (D, D), BF16), ("wv", (D, D), BF16),
        ("w1", (D, DFF), BF16), ("w2", (DFF, D), BF16),
        ("bq", (D,), np.float32), ("bk", (D,), np.float32),
        ("bv", (D,), np.float32), ("b1", (DFF,), np.float32),
        ("b2", (D,), np.float32),
        ("fng", (D,), np.float32), ("fnb", (D,), np.float32),
        ("cq2", (128, T), BF16), ("sq2", (128, T), BF16), ("csq2", (128, T), BF16),
        ("ck2", (128, M), BF16), ("sk2", (128, M), BF16), ("csk2", (128, M), BF16),
        ("bo16", (128, 8 * 16), BF16),
        ("indall", (16, D), BF16),
        ("perm", (128, 128), BF16),
        ("id64", (64, 64), BF16),
        ("onesb", (1, 128), np.float32),     # fp32 ones row (recip broadcast lhsT)
        ("ones128f", (128, 1), np.float32),  # value 1/1024 (FFN stats lhsT)
        ("onesr", (1, 128), BF16),           # bf16 ones row (FFN bc lhsT)
    ]
    if with_tq:
        sp.append(("tq", (128, T), BF16))
    if with_tk:
        sp.append(("tk", (128, M), BF16))
    return sp


def build(with_tq=False, with_tk=False):
    import concourse.bass as bass
    import concourse.mybir as mybir
    import concourse.tile as tile
    from contextlib import ExitStack

    _patch_tile_drain()
    AF = mybir.ActivationFunctionType
    OP = mybir.AluOpType
    bf = mybir.dt.bfloat16
    f32 = mybir.dt.float32

    nc = bass.Bass()
    dram = {}
    for name, shape, npdt in input_specs(with_tq, with_tk):
        dt = bf if npdt is BF16 else f32
        dram[name] = nc.declare_dram_parameter(name, list(shape), dt, isOutput=False)
    out_d = nc.declare_dram_parameter("out", [D, T], f32, isOutput=True)

    with tile.TileContext(nc) as tc, ExitStack() as ctx:
        const = ctx.enter_context(tc.tile_pool(name="const", bufs=1))
        u = ctx.enter_context(tc.tile_pool(name="u", bufs=1))
        wt = ctx.enter_context(tc.tile_pool(name="wt", bufs=3))
        pmm = ctx.enter_context(tc.tile_pool(name="pmm", bufs=4, space="PSUM"))
        pstat = ctx.enter_context(tc.tile_pool(name="pstat", bufs=2, space="PSUM"))
        pot = ctx.enter_context(tc.tile_pool(name="pot", bufs=2, space="PSUM"))

        def load(pool, name, rearr=None, tag=None, **kw):
            src = dram[name][:]
            if rearr is not None:
                src = src.rearrange(rearr, **kw)
            t = pool.tile(list(src.shape), src.dtype, tag=tag or name, name=name)
            nc.sync.dma_start(out=t[:], in_=src)
            return t

        # small constants
        bo16 = load(const, "bo16")
        indall = load(const, "indall")
        perm = load(const, "perm")
        id64 = load(const, "id64")
        onesb = load(const, "onesb")
        ones128f = load(const, "ones128f")
        onesr = load(const, "onesr")
        bq_sb = load(const, "bq", "(c p) -> p c", p=128)
        bk_sb = load(const, "bk", "(c p) -> p c", p=128)
        bv_sb = load(const, "bv", "(c p) -> p c", p=128)
        b2_sb = load(const, "b2", "(c p) -> p c", p=128)
        b1_sb = load(const, "b1", "(c p) -> p c", p=128)
        fng_sb = load(const, "fng", "(c p) -> p c", p=128)
        fnb_sb = load(const, "fnb", "(c p) -> p c", p=128)
        eps = const.tile([128, 1], f32, tag="eps", name="eps")
        nc.vector.memset(eps[:], 1e-5)

        # rope tables — their tag slots are later reused by attention/FFN tiles
        cq2 = load(u, "cq2"); sq2 = load(u, "sq2"); csq2 = load(u, "csq2")
        ck2 = load(u, "ck2"); sk2 = load(u, "sk2"); csk2 = load(u, "csk2")
        tqt = load(const, "tq") if with_tq else None
        tkt = load(const, "tk") if with_tk else None

        # big activations (tags are temporal chains — comments show the chain)
        xc_sb = u.tile([128, NCH, M], bf, tag="cA", name="xc")       # cA: xc->xqf->h1
        nc.sync.dma_start(out=xc_sb[:], in_=dram["xc"][:].rearrange("(c p) m -> p c m", p=128))
        xq_sb = u.tile([128, NCH, T], bf, tag="cE", name="xqbf")     # cE: xq->h
        nc.sync.dma_start(out=xq_sb[:], in_=dram["xq"][:].rearrange("(c p) n -> p c n", p=128))
        wv_sb = u.tile([128, NCH, D], bf, tag="cD", name="wvsb")     # cD: wv->OT
        nc.sync.dma_start(out=wv_sb[:], in_=dram["wv"][:].rearrange("(k p) m -> p k m", p=128))

        KT = u.tile([128, NCH, M], bf, tag="cB", name="KT")          # cB: KT->o
        QT = u.tile([128, NCH, T], bf, tag="cQT", name="QT")
        V = u.tile([128, MC, H, DH + 1], bf, tag="cC", name="V")     # cC: V->x2f
        aK = u.tile([16, M], bf, tag="caK", name="aK")               # caK: aK->dsb
        bK = u.tile([16, M], bf, tag="cbK", name="bK")               # cbK: bK->rsb
        aQ = u.tile([16, T], bf, tag="caQ", name="aQ")               # caQ: aQ->af
        bQ = u.tile([16, T], bf, tag="cbQ", name="bQ")               # cbQ: bQ->bff

        # V ones-columns (col DH for every head)
        nc.vector.memset(V[:, :, :, DH:DH + 1], 1.0)

        # ---------------- projections ----------------
        def proj_featmajor(wname, x_sb, ntt, out_t, bias):
            # kc inner over n so each weight chunk (lhsT) is reused across the
            # ntt moving tiles — amortizes the serial LDWEIGHTS
            wr = dram[wname][:].rearrange("(k p) m -> p k m", p=128)
            for m in range(NCH):
                wtile = wt.tile([128, NCH, 128], bf, tag="wqk", name=f"w_{wname}_{m}")
                nc.sync.dma_start(out=wtile[:], in_=wr[:, :, m * 128:(m + 1) * 128])
                pss = [pmm.tile([128, 512], f32, tag="pmm", name=f"ps_{wname}_{m}_{n}")
                       for n in range(ntt)]
                for kc in range(NCH):
                    for n in range(ntt):
                        nc.tensor.matmul(pss[n][:], lhsT=wtile[:, kc, :],
                                         rhs=x_sb[:, kc, n * 512:(n + 1) * 512],
                                         start=(kc == 0), stop=(kc == NCH - 1),
                                         skip_group_check=True)
                for n in range(ntt):
                    nc.scalar.activation(out=out_t[:, m, n * 512:(n + 1) * 512],
                                         in_=pss[n][:], func=AF.Identity,
                                         bias=bias[:, m:m + 1], scale=1.0)

        proj_featmajor("wk", xc_sb, TTK, KT, bk_sb)
        proj_featmajor("wq", xq_sb, 1, QT, bq_sb)

        # V token-major (no bias: the V bias is exactly additive after softmax)
        for mc in range(MC):
            psa = pmm.tile([128, 512], f32, tag="pmm", name=f"ps_v_{mc}_0")
            psb = pmm.tile([128, 512], f32, tag="pmm", name=f"ps_v_{mc}_1")
            for kc in range(NCH):
                lh = xc_sb[:, kc, mc * 128:(mc + 1) * 128]
                nc.tensor.matmul(psa[:], lhsT=lh, rhs=wv_sb[:, kc, 0:512],
                                 start=(kc == 0), stop=(kc == NCH - 1),
                                 skip_group_check=True)
                nc.tensor.matmul(psb[:], lhsT=lh, rhs=wv_sb[:, kc, 512:1024],
                                 start=(kc == 0), stop=(kc == NCH - 1),
                                 skip_group_check=True)
            for n, ps in enumerate((psa, psb)):
                pv = ps[:].rearrange("p (hh d) -> p hh d", d=DH)
                nc.scalar.activation(out=V[:, mc, 8 * n:8 * n + 8, 0:DH],
                                     in_=pv[:], func=AF.Copy)

        # xqf (residual) loads into the xc slot once projections are done;
        # OT takes over wv's slot
        xqf_sb = u.tile([128, NCH, T], f32, tag="cA", name="xqf")
        nc.sync.dma_start(out=xqf_sb[:], in_=dram["xqf"][:].rearrange("(c p) n -> p c n", p=128))
        OT = u.tile([128, NCH, T], bf, tag="cD", name="OT")

        # ---------------- per-head QK layernorm + rope ----------------
        def ln_stats(X, ntt, a_sb, b_sb, pref):
            for tt in range(ntt):
                ts_ = slice(tt * 512, (tt + 1) * 512)
                sx = pstat.tile([16, 512], f32, tag="pstat", name=f"sx_{pref}{tt}")
                sq = pstat.tile([16, 512], f32, tag="pstat", name=f"sq_{pref}{tt}")
                for c in range(NCH):
                    xs = X[:, c, ts_]
                    x2 = u.tile([128, 512], bf, tag="x2", bufs=2, name=f"x2_{pref}{tt}_{c}")
                    nc.vector.tensor_mul(out=x2[:], in0=xs, in1=xs)
                    # one-hot block lhsT accumulates chunk c's two head rows;
                    # the other 14 rows accumulate zeros
                    nc.tensor.matmul(sx[:], lhsT=bo16[:, c * 16:(c + 1) * 16], rhs=xs,
                                     start=(c == 0), stop=(c == NCH - 1),
                                     skip_group_check=True)
                    nc.tensor.matmul(sq[:], lhsT=bo16[:, c * 16:(c + 1) * 16], rhs=x2[:],
                                     start=(c == 0), stop=(c == NCH - 1),
                                     skip_group_check=True)
                mu = u.tile([16, 512], f32, tag="cmu", name=f"mu_{pref}{tt}")
                nc.scalar.activation(out=mu[:], in_=sx[:], func=AF.Copy)
                t1 = u.tile([16, 512], f32, tag="ct1", name=f"t1_{pref}{tt}")
                nc.vector.tensor_mul(out=t1[:], in0=mu[:], in1=mu[:])
                var = u.tile([16, 512], f32, tag="cvar", name=f"var_{pref}{tt}")
                nc.vector.tensor_tensor(out=var[:], in0=sq[:], in1=t1[:], op=OP.subtract)
                sd = u.tile([16, 512], f32, tag="csd", name=f"sd_{pref}{tt}")
                nc.scalar.activation(out=sd[:], in_=var[:], func=AF.Sqrt,
                                     bias=eps[0:16, :], scale=1.0)
                with nc.allow_low_precision("bf16 rstd for broadcast matmul"):
                    nc.vector.reciprocal(out=a_sb[:, ts_], in_=sd[:])
                nc.vector.tensor_mul(out=b_sb[:, ts_], in0=mu[:], in1=a_sb[:, ts_])

        def ln_rope(X, ntt, a_sb, b_sb, ctab, stab, cstab, ttab, pref):
            Nw = ntt * 512
            for c in range(NCH):
                zz1 = u.tile([128, M], bf, tag="czz1", name=f"zz1_{pref}{c}")
                zz2 = u.tile([128, M], bf, tag="czz2", name=f"zz2_{pref}{c}")
                A = u.tile([128, M], bf, tag="cAbc", name=f"A_{pref}{c}")
                Bt = u.tile([128, M], bf, tag="cBbc", name=f"B_{pref}{c}")
                for tt in range(ntt):
                    ts_ = slice(tt * 512, (tt + 1) * 512)
                    aps = pmm.tile([128, 512], f32, tag="pmm", name=f"aps_{pref}{c}_{tt}")
                    nc.tensor.matmul(aps[:], lhsT=indall[:, c * 128:(c + 1) * 128],
                                     rhs=a_sb[:, ts_], start=True, stop=True)
                    nc.scalar.activation(out=A[:, ts_], in_=aps[:], func=AF.Copy)
                    bps = pmm.tile([128, 512], f32, tag="pmm", name=f"bps_{pref}{c}_{tt}")
                    nc.tensor.matmul(bps[:], lhsT=indall[:, c * 128:(c + 1) * 128],
                                     rhs=b_sb[:, ts_], start=True, stop=True)
                    nc.scalar.activation(out=Bt[:, ts_], in_=bps[:], func=AF.Copy)
                    rot = pmm.tile([128, 512], f32, tag="pmm", name=f"rot_{pref}{c}_{tt}")
                    nc.tensor.matmul(rot[:], lhsT=perm[:], rhs=X[:, c, ts_],
                                     start=True, stop=True)
                    nc.vector.tensor_mul(out=zz2[:, ts_], in0=rot[:], in1=stab[:, ts_])
                # out = A*(C2*x + S2*rot) - B*(C2+S2) [+ Tadd]
                nc.vector.tensor_mul(out=zz1[:, :Nw], in0=X[:, c, :Nw], in1=ctab[:, :Nw])
                nc.vector.tensor_add(out=zz1[:, :Nw], in0=zz1[:, :Nw], in1=zz2[:, :Nw])
                nc.vector.tensor_mul(out=zz1[:, :Nw], in0=zz1[:, :Nw], in1=A[:, :Nw])
                nc.vector.tensor_mul(out=zz2[:, :Nw], in0=Bt[:, :Nw], in1=cstab[:, :Nw])
                nc.vector.tensor_tensor(out=X[:, c, :Nw], in0=zz1[:, :Nw],
                                        in1=zz2[:, :Nw], op=OP.subtract)
                if ttab is not None:
                    nc.vector.tensor_add(out=X[:, c, :Nw], in0=X[:, c, :Nw],
                                         in1=ttab[:, :Nw])

        ln_stats(KT, TTK, aK, bK, "k")
        ln_rope(KT, TTK, aK, bK, ck2, sk2, csk2, tkt, "k")
        ln_stats(QT, 1, aQ, bQ, "q")
        ln_rope(QT, 1, aQ, bQ, cq2, sq2, csq2, tqt, "q")

        # ---------------- attention ----------------
        # Head pairs (even at partitions 0..63, odd at 64..127 of their chunk)
        # interleave their K=64 QK matmuls so LDWEIGHTS for the other row-group
        # overlaps the running matmul. Denominators from all 16 heads are
        # gathered into one [16, 512] tile; a single reciprocal + per-chunk
        # indicator-broadcast normalizes OT at the end.
        WV = 4                      # ctx chunks per wave
        att_tags = ["ck2", "sk2", "csk2", "catt"]
        dall = u.tile([16, 512], f32, tag="cdall", name="dall")
        for pair in range(NCH):
            he, ho = 2 * pair, 2 * pair + 1
            c = pair
            ote = pot.tile([128, 512], f32, tag="pot", name=f"ot_{he}")
            oto = pot.tile([128, 512], f32, tag="pot", name=f"ot_{ho}")
            for w in range(MC // WV):
                atte = u.tile([128, WV, 512], bf, tag=att_tags[w % 2],
                              name=f"atte_{pair}_{w}")
                atto = u.tile([128, WV, 512], bf, tag=att_tags[2 + w % 2],
                              name=f"atto_{pair}_{w}")
                for i in range(WV):
                    mc = w * WV + i
                    spe = pmm.tile([128, 512], f32, tag="pmm", name=f"spe_{pair}_{mc}")
                    nc.tensor.matmul(spe[:], lhsT=KT[0:64, c, mc * 128:(mc + 1) * 128],
                                     rhs=QT[0:64, c, :], start=True, stop=True)
                    spo = pmm.tile([128, 512], f32, tag="pmm", name=f"spo_{pair}_{mc}")
                    nc.tensor.matmul(spo[:], lhsT=KT[64:128, c, mc * 128:(mc + 1) * 128],
                                     rhs=QT[64:128, c, :], start=True, stop=True)
                    nc.scalar.activation(out=atte[:, i, :], in_=spe[:], func=AF.Exp,
                                         scale=0.125)
                    nc.scalar.activation(out=atto[:, i, :], in_=spo[:], func=AF.Exp,
                                         scale=0.125)
                    nc.tensor.matmul(ote[0:DH + 1, :], lhsT=V[:, mc, he, :],
                                     rhs=atte[:, i, :], start=(mc == 0),
                                     stop=(mc == MC - 1), skip_group_check=True)
                    nc.tensor.matmul(oto[0:DH + 1, :], lhsT=V[:, mc, ho, :],
                                     rhs=atto[:, i, :], start=(mc == 0),
                                     stop=(mc == MC - 1), skip_group_check=True)
            # stash unnormalized O and the denominators
            nc.scalar.activation(out=OT[0:64, c, :], in_=ote[0:64, :], func=AF.Copy)
            dsb = u.tile([128, 512], f32, tag="caK", name=f"dsb_{he}")
            nc.scalar.activation(out=dsb[64:65, :], in_=ote[64:65, :], func=AF.Copy)
            nc.sync.dma_start(out=dall[he:he + 1, :], in_=dsb[64:65, :])
            # odd head: O sits at PSUM rows 0..63 but belongs at partitions
            # 64..127 of OT; shift with an identity matmul (PE can cross
            # partitions, DVE/ACT cannot)
            tmp = u.tile([128, 512], bf, tag="cotmp", bufs=2, name=f"otmp_{ho}")
            nc.scalar.activation(out=tmp[0:64, :], in_=oto[0:64, :], func=AF.Copy)
            ps2 = pmm.tile([128, 512], f32, tag="pmm", name=f"ps2_{ho}")
            nc.tensor.matmul(ps2[64:128, :], lhsT=id64[:], rhs=tmp[0:64, :],
                             start=True, stop=True)
            nc.scalar.activation(out=OT[64:128, c, :], in_=ps2[64:128, :], func=AF.Copy)
            dsb2 = u.tile([128, 512], f32, tag="caK", name=f"dsb_{ho}")
            nc.scalar.activation(out=dsb2[64:65, :], in_=oto[64:65, :], func=AF.Copy)
            nc.sync.dma_start(out=dall[ho:ho + 1, :], in_=dsb2[64:65, :])
        # one reciprocal for all heads, broadcast per chunk, normalize in place
        rall = u.tile([16, 512], bf, tag="crall", name="rall")
        with nc.allow_low_precision("bf16 softmax reciprocal broadcast"):
            nc.vector.reciprocal(out=rall[:], in_=dall[:])
        for c in range(NCH):
            rps = pmm.tile([128, 512], f32, tag="pmm", name=f"rps_{c}")
            nc.tensor.matmul(rps[:], lhsT=indall[:, c * 128:(c + 1) * 128],
                             rhs=rall[:], start=True, stop=True)
            rsb = u.tile([128, 512], bf, tag="cbK", name=f"rsb_{c}")
            nc.scalar.activation(out=rsb[:], in_=rps[:], func=AF.Copy)
            nc.vector.tensor_mul(out=OT[:, c, :], in0=OT[:, c, :], in1=rsb[:])

        # ---------------- residual + FFN ----------------
        o_sb = u.tile([128, NCH, T], f32, tag="cB", name="o")
        for c in range(NCH):
            nc.vector.tensor_add(out=o_sb[:, c, :], in0=xqf_sb[:, c, :], in1=OT[:, c, :])
            nc.vector.tensor_scalar_add(out=o_sb[:, c, :], in0=o_sb[:, c, :],
                                        scalar1=bv_sb[:, c:c + 1])

        # FFN layernorm over all 1024 features: fp32 ones-matmul stats
        x2f = u.tile([128, NCH, T], f32, tag="cC", name="x2f")
        for c in range(NCH):
            nc.vector.tensor_mul(out=x2f[:, c, :], in0=o_sb[:, c, :], in1=o_sb[:, c, :])
        smean = pstat.tile([16, 512], f32, tag="pstat", name="smean")
        for c in range(NCH):
            nc.tensor.matmul(smean[0:1, :], lhsT=ones128f[:], rhs=o_sb[:, c, :],
                             start=(c == 0), stop=(c == NCH - 1))
        smsq = pstat.tile([16, 512], f32, tag="pstat", name="smsq")
        for c in range(NCH):
            nc.tensor.matmul(smsq[0:1, :], lhsT=ones128f[:], rhs=x2f[:, c, :],
                             start=(c == 0), stop=(c == NCH - 1))
        muf = u.tile([1, 512], f32, tag="cmu", name="muf")
        nc.scalar.activation(out=muf[:], in_=smean[0:1, :], func=AF.Copy)
        t1f = u.tile([1, 512], f32, tag="ct1", name="t1f")
        nc.vector.tensor_mul(out=t1f[:], in0=muf[:], in1=muf[:])
        varf = u.tile([1, 512], f32, tag="cvar", name="varf")
        nc.vector.tensor_tensor(out=varf[:], in0=smsq[0:1, :], in1=t1f[:], op=OP.subtract)
        sdf = u.tile([1, 512], f32, tag="csd", name="sdf")
        nc.scalar.activation(out=sdf[:], in_=varf[:], func=AF.Sqrt, bias=eps[0:1, :],
                             scale=1.0)
        af = u.tile([1, 512], bf, tag="caQ", name="af")
        with nc.allow_low_precision("bf16 rstd for broadcast matmul"):
            nc.vector.reciprocal(out=af[:], in_=sdf[:])
        bff = u.tile([1, 512], bf, tag="cbQ", name="bff")
        nc.vector.tensor_mul(out=bff[:], in0=muf[:], in1=af[:])
        a2ps = pmm.tile([128, 512], f32, tag="pmm", name="a2ps")
        nc.tensor.matmul(a2ps[:], lhsT=onesr[:], rhs=af[:], start=True, stop=True)
        A2 = u.tile([128, 512], bf, tag="cq2", name="A2")
        nc.scalar.activation(out=A2[:], in_=a2ps[:], func=AF.Copy)
        b2ps = pmm.tile([128, 512], f32, tag="pmm", name="b2ps")
        nc.tensor.matmul(b2ps[:], lhsT=onesr[:], rhs=bff[:], start=True, stop=True)
        B2 = u.tile([128, 512], bf, tag="sq2", name="B2")
        nc.scalar.activation(out=B2[:], in_=b2ps[:], func=AF.Copy)

        h_sb = u.tile([128, NCH, T], bf, tag="cE", name="hsb")
        for c in range(NCH):
            tn = u.tile([128, 512], f32, tag="csk2", name=f"tn_{c}")
            nc.vector.tensor_mul(out=tn[:], in0=o_sb[:, c, :], in1=A2[:])
            nc.vector.tensor_tensor(out=tn[:], in0=tn[:], in1=B2[:], op=OP.subtract)
            nc.vector.tensor_scalar(out=h_sb[:, c, :], in0=tn[:],
                                    scalar1=fng_sb[:, c:c + 1],
                                    scalar2=fnb_sb[:, c:c + 1],
                                    op0=OP.mult, op1=OP.add)

        # FFN matmul 1 + exact GELU
        h1_sb = u.tile([128, DFF // 128, T], bf, tag="cA", name="h1")
        w1r = dram["w1"][:].rearrange("(k p) m -> p k m", p=128)
        for m in range(DFF // 128):
            w1t = wt.tile([128, NCH, 128], bf, tag="wqk", name=f"w1t_{m}")
            nc.sync.dma_start(out=w1t[:], in_=w1r[:, :, m * 128:(m + 1) * 128])
            ps = pmm.tile([128, 512], f32, tag="pmm", name=f"ps_h1_{m}")
            for kc in range(NCH):
                nc.tensor.matmul(ps[:], lhsT=w1t[:, kc, :], rhs=h_sb[:, kc, :],
                                 start=(kc == 0), stop=(kc == NCH - 1))
            nc.scalar.activation(out=h1_sb[:, m, :], in_=ps[:], func=AF.Gelu,
                                 bias=b1_sb[:, m:m + 1], scale=1.0)

        # FFN matmul 2 + bias + residual (w2 streamed as two half-K tiles that
        # reuse the ck2/csk2 table slots)
        w2r = dram["w2"][:].rearrange("(k p) m -> p k m", p=128)
        KH = DFF // 128 // 2        # 16 k-chunks per half
        for m in range(NCH):
            w2a = u.tile([128, KH, 128], bf, tag="ck2", name=f"w2a_{m}")
            nc.sync.dma_start(out=w2a[:], in_=w2r[:, 0:KH, m * 128:(m + 1) * 128])
            w2b = u.tile([128, KH, 128], bf, tag="csk2", name=f"w2b_{m}")
            nc.sync.dma_start(out=w2b[:], in_=w2r[:, KH:2 * KH, m * 128:(m + 1) * 128])
            ps = pmm.tile([128, 512], f32, tag="pmm", name=f"ps_h2_{m}")
            for kc in range(2 * KH):
                wsl = w2a[:, kc, :] if kc < KH else w2b[:, kc - KH, :]
                nc.tensor.matmul(ps[:], lhsT=wsl, rhs=h1_sb[:, kc, :],
                                 start=(kc == 0), stop=(kc == 2 * KH - 1))
            nc.vector.tensor_add(out=o_sb[:, m, :], in0=ps[:], in1=o_sb[:, m, :])
            nc.vector.tensor_scalar_add(out=o_sb[:, m, :], in0=o_sb[:, m, :],
                                        scalar1=b2_sb[:, m:m + 1])
            nc.sync.dma_start(
                out=out_d[:].rearrange("(c p) n -> p c n", p=128)[:, m, :],
                in_=o_sb[:, m, :])

    _split_sync_waits(nc)
    return nc


# ---------------------------------------------------------------- host side

def _rope_tables(pos, g, b_ln):
    """Feature-major rope coefficient tiles [128, N] (pattern repeats per 64).

    out = C2*z + S2*rot(z) + Tadd with z the per-head layernormed vector,
    C2 = C*G[p], S2 = S*G[rp], Tadd = C*B[p] + S*B[rp].
    """
    half = DH // 2
    inv = (1.0 / (10000.0 ** (np.arange(half, dtype=np.float32) / half))).astype(np.float32)
    ang = pos.astype(np.float32)[None, :] * inv[:, None]          # [32, N]
    c = np.cos(ang).astype(np.float32)
    s = np.sin(ang).astype(np.float32)
    C64 = np.concatenate([c, c], axis=0)                          # [64, N]
    S64 = np.concatenate([-s, s], axis=0)
    G = np.ones(DH, np.float32) if g is None else np.asarray(g, np.float32)
    Bv = np.zeros(DH, np.float32) if b_ln is None else np.asarray(b_ln, np.float32)
    rp = np.concatenate([np.arange(32, 64), np.arange(0, 32)])
    C2 = C64 * G[:, None]
    S2 = S64 * G[rp][:, None]
    CS2 = C2 + S2
    Tadd = C64 * Bv[:, None] + S64 * Bv[rp][:, None]
    tile = lambda X: np.concatenate([X, X], axis=0)               # [128, N]
    has_t = bool(np.abs(Bv).max() > 0)
    return (tile(C2).astype(BF16), tile(S2).astype(BF16), tile(CS2).astype(BF16),
            tile(Tadd).astype(BF16) if has_t else None)


def _consts():
    bo16 = np.zeros((128, 8, 16), np.float32)
    for c in range(NCH):
        for pp in range(128):
            bo16[pp, c, 2 * c + (pp >= 64)] = 1.0 / DH
    bo16 = bo16.reshape(128, 8 * 16)
    indall = np.zeros((16, D), np.float32)
    for c in range(NCH):
        for pp in range(128):
            indall[2 * c + (pp >= 64), c * 128 + pp] = 1.0
    perm = np.zeros((128, 128), np.float32)
    for mm in range(128):
        k = (mm // 64) * 64 + ((mm % 64) + 32) % 64
        perm[k, mm] = 1.0
    return {
        "bo16": bo16.astype(BF16),
        "indall": indall.astype(BF16),
        "perm": perm.astype(BF16),
        "id64": np.eye(64, dtype=np.float32).astype(BF16),
        "onesb": np.ones((1, 128), np.float32),
        "ones128f": np.full((128, 1), 1.0 / D, np.float32),
        "onesr": np.ones((1, 128), BF16),
    }


def make_in_maps(inputs):
    """Full inputs -> (per-core input dicts, build flags)."""
    inputs = {k: np.asarray(v) for k, v in inputs.items()}
    consts = _consts()
    shared = {
        "wq": inputs["Wq"].astype(BF16), "wk": inputs["Wk"].astype(BF16),
        "wv": inputs["Wv"].astype(BF16), "w1": inputs["W1"].astype(BF16),
        "w2": inputs["W2"].astype(BF16),
        "bq": inputs["bq"].astype(np.float32), "bk": inputs["bk"].astype(np.float32),
        "bv": inputs["bv"].astype(np.float32), "b1": inputs["b1"].astype(np.float32),
        "b2": inputs["b2"].astype(np.float32),
        "fng": inputs["fn_g"].astype(np.float32), "fnb": inputs["fn_b"].astype(np.float32),
        **consts,
    }
    in_maps = []
    with_tq = with_tk = False
    for core in range(8):
        b, t0 = core // 4, (core % 4) * T
        xqf = np.ascontiguousarray(inputs["query"][b, t0:t0 + T].T).astype(np.float32)
        cq, sq, csq, tq = _rope_tables(inputs["qpos"][b, t0:t0 + T],
                                       inputs["qn_g"], inputs["qn_b"])
        ck, sk, csk, tk = _rope_tables(inputs["cpos"][b],
                                       inputs["kn_g"], inputs["kn_b"])
        m = dict(shared)
        m.update({
            "xqf": xqf, "xq": xqf.astype(BF16),
            "xc": np.ascontiguousarray(inputs["context"][b].T).astype(BF16),
            "cq2": cq, "sq2": sq, "csq2": csq,
            "ck2": ck, "sk2": sk, "csk2": csk,
        })
        if tq is not None:
            m["tq"] = tq
            with_tq = True
        if tk is not None:
            m["tk"] = tk
            with_tk = True
        in_maps.append(m)
    return in_maps, with_tq, with_tk


def kernel(**inputs):
    from concourse.bass_utils import run_bass_kernel_spmd
    in_maps, with_tq, with_tk = make_in_maps(inputs)
    key = (with_tq, with_tk)
    if key not in _BUILT:
        _BUILT[key] = build(*key)
    nc = _BUILT[key]
    res = run_bass_kernel_spmd(nc, in_maps, core_ids=list(range(8)))
    out = np.zeros((B, N, D), np.float32)
    for core in range(8):
        b, t0 = core // 4, (core % 4) * T
        out[b, t0:t0 + T] = res.results[core]["out"].T
    return out
